# revision 1
# baseline (speedup 1.0000x reference)
"""RWKV v4 block (nn_Block_15109694947416) on 8 TRN2 NeuronCores.

Strategy:
- Data-parallel over B: core i processes batch i (B=8). No collectives.
- Activations live channel-major [C, T] on-chip: matmuls contract over the
  partition dim natively (lhsT = weight in its DRAM layout), the token-shift
  is a free-dim offset, and the WKV recurrence maps onto the hardware
  tensor_tensor_scan (state = ew*state + data) along the free dim.
- Host pre-transposes x[b] -> [C, T] and precomputes ew=exp(-exp(time_decay)),
  eu=exp(time_first). WKV is computed unstabilized in fp32 which is exact
  (validated ~1e-6 vs the stabilized reference): k is small, w<0.
- time-mix matmuls in float32r (1 cyc/row at N>=256, ~1.4e-4 rel err),
  FFN matmuls in bf16 (~2e-3 on a small additive branch).
- T processed in 8 chunks of 256 columns; scan/token-shift state carried
  across chunks via [128, 8, 1] carry tiles.
"""

import numpy as np
import ml_dtypes

B, T, C = 8, 2048, 1024
TC = 256                 # time chunk
NCH = T // TC            # chunks
CB = C // 128            # channel blocks (8)
FB = 4 * C // 128        # ffn hidden blocks (32)
EPS = 1e-5

_CACHE = {}


def _bcast_free(ap, n):
    """[128,1] AP -> [128,n] stride-0 broadcast along free dim."""
    import concourse.bass as bass
    return bass.AP(tensor=ap.tensor, offset=ap.offset, ap=[ap.ap[0], [0, n]])


def _bcast_mid(ap, nmid):
    """[128,N] AP -> [128,nmid,N] stride-0 broadcast of a middle dim."""
    import concourse.bass as bass
    return bass.AP(tensor=ap.tensor, offset=ap.offset,
                   ap=[ap.ap[0], [0, nmid], ap.ap[1]])


def _build():
    import concourse.bass as bass
    import concourse.bacc as bacc
    import concourse.tile as tile
    from concourse import mybir

    f32 = mybir.dt.float32
    f32r = mybir.dt.float32r
    bf16 = mybir.dt.bfloat16
    AF = mybir.ActivationFunctionType
    OP = mybir.AluOpType

    nc = bacc.Bacc(None, target_bir_lowering=False, debug=False)

    xT = nc.dram_tensor("xT", [C, T], f32r, kind="ExternalInput")
    cvecs = nc.dram_tensor("cvecs", [128, CB, 12], f32, kind="ExternalInput")
    ones_in = nc.dram_tensor("ones128", [128], f32r, kind="ExternalInput")
    Wk = nc.dram_tensor("Wk", [C, C], f32r, kind="ExternalInput")
    Wv = nc.dram_tensor("Wv", [C, C], f32r, kind="ExternalInput")
    Wr = nc.dram_tensor("Wr", [C, C], f32r, kind="ExternalInput")
    Wo = nc.dram_tensor("Wo", [C, C], bf16, kind="ExternalInput")
    fWk = nc.dram_tensor("fWk", [C, 4 * C], bf16, kind="ExternalInput")
    fWv = nc.dram_tensor("fWv", [4 * C, C], bf16, kind="ExternalInput")
    fWr = nc.dram_tensor("fWr", [C, C], bf16, kind="ExternalInput")
    outT = nc.dram_tensor("outT", [C, T], f32, kind="ExternalOutput")


    def dma8(out_t, in_ap, parts=8):
        """Split a [128, M, N] transfer along the middle dim across DMA queues."""
        M = out_t.shape[1]
        step = max(1, M // parts)
        for i in range(0, M, step):
            j = min(i + step, M)
            nc.sync.dma_start(out=out_t[:, i:j, :], in_=in_ap[:, i:j, :])


    def dma8_out(dram_ap, sb_t, parts=8):
        M = sb_t.shape[1]
        step = max(1, M // parts)
        for i in range(0, M, step):
            j = min(i + step, M)
            nc.sync.dma_start(out=dram_ap[:, i:j, :], in_=sb_t[:, i:j, :])

    # cvec row indices
    LN1G, LN1B, LN2G, LN2B, TMK, TMV, TMR, FTMK, FTMR, EW, EU, _ = range(12)

    with tile.TileContext(nc) as tc:
        import contextlib
        with contextlib.ExitStack() as ctx:
            consts = ctx.enter_context(tc.tile_pool(name="consts", bufs=1))
            dramp = ctx.enter_context(tc.tile_pool(name="dram", bufs=1, space="DRAM"))

            cv = consts.tile([128, CB, 12], f32)
            nc.sync.dma_start(out=cv, in_=cvecs[:, :, :])
            ones_k = consts.tile([128, 1], f32r)    # lhsT for column sums
            nc.sync.dma_start(out=ones_k, in_=ones_in.rearrange("(p o) -> p o", o=1))
            ones_b = consts.tile([1, 128], f32r)    # lhsT for row broadcast
            nc.sync.dma_start(out=ones_b, in_=ones_in.rearrange("(o p) -> o p", o=1))

            eps_t = consts.tile([1, 1], f32)
            nc.vector.memset(eps_t, EPS)
            carryH = consts.tile([128, CB, 1], f32)
            carryG = consts.tile([128, CB, 1], f32)
            carryA = consts.tile([128, CB, 1], f32)
            carryB = consts.tile([128, CB, 1], f32)
            for c in (carryH, carryG, carryA, carryB):
                nc.vector.memset(c, 0.0)

            x2d = dramp.tile([NCH, 128, CB, TC], f32)

            def layernorm(pools, x_t, g_row, b_row, h_t):
                """x_t: [128, CB, TC] f32r tile -> h_t[:, :, 1:TC+1] fp32.

                Per-token stats via PE ones-matmuls (cross-partition sums),
                broadcast back via K=1 matmuls.
                """
                sbuf, ps_stat, ps_bc, scratch, sq_tag, s1_tag = pools
                sq = sbuf.tile([128, CB, TC], f32r, tag=sq_tag)
                nc.scalar.activation(out=sq, in_=x_t.bitcast(f32), func=AF.Square)
                st = ps_stat.tile([1, 2 * TC], f32)
                for cb in range(CB):
                    nc.tensor.matmul(st[:, 0:TC], ones_k, x_t[:, cb, :],
                                     start=(cb == 0), stop=(cb == CB - 1))
                for cb in range(CB):
                    nc.tensor.matmul(st[:, TC:2 * TC], ones_k, sq[:, cb, :],
                                     start=(cb == 0), stop=(cb == CB - 1))
                rows = scratch.tile([1, 2 * TC], f32r, tag="rows")
                tmp = scratch.tile([1, 2 * TC], f32, tag="rtmp")
                rowf = rows.bitcast(f32)
                # m = sum/C  (f32r-typed out: consumed by broadcast matmul)
                nc.vector.tensor_scalar_mul(rows[:, 0:TC], st[:, 0:TC], 1.0 / C)
                # m^2
                nc.vector.tensor_mul(tmp[:, 0:TC], rowf[:, 0:TC], rowf[:, 0:TC])
                # var = sumsq/C - m^2
                nc.vector.scalar_tensor_tensor(
                    out=tmp[:, TC:2 * TC], in0=st[:, TC:2 * TC], scalar=1.0 / C,
                    in1=tmp[:, 0:TC], op0=OP.mult, op1=OP.subtract)
                # rstd = 1/sqrt(var + eps)
                nc.scalar.activation(out=tmp[:, TC:2 * TC], in_=tmp[:, TC:2 * TC],
                                     func=AF.Sqrt, bias=eps_t[:, :])
                nc.vector.reciprocal_approx_fast(out=tmp[:, 0:TC],
                                                 in_=tmp[:, TC:2 * TC])
                nc.vector.tensor_copy(out=rows[:, TC:2 * TC], in_=tmp[:, 0:TC])
                # broadcast m and rstd across partitions
                mb = ps_bc.tile([128, TC], f32, tag="mb")
                nc.tensor.matmul(mb, ones_b, rows[:, 0:TC])
                rb = ps_bc.tile([128, TC], f32, tag="rb")
                nc.tensor.matmul(rb, ones_b, rows[:, TC:2 * TC])
                s1 = sbuf.tile([128, CB, TC], f32, tag=s1_tag)
                nc.vector.tensor_sub(s1, x_t.bitcast(f32), _bcast_mid(mb, CB))
                nc.vector.tensor_mul(s1, s1, _bcast_mid(rb, CB))
                for cb in range(CB):
                    nc.scalar.activation(
                        out=h_t[:, cb, 1:TC + 1], in_=s1[:, cb, :],
                        func=AF.Identity, bias=b_row(cb), scale=g_row(cb))

            # ================= Phase 1a: time-mix k/v/r =================
            import os
            _PH = os.environ.get("KPHASES", "12")
            ekd = dramp.tile([NCH, 128, CB, TC], f32, tag="ekd")
            ekvd = dramp.tile([NCH, 128, CB, TC], f32, tag="ekvd")
            rsd1 = dramp.tile([NCH, 128, CB, TC], bf16, tag="rsd1")
            if "1" in _PH:
              with contextlib.ExitStack() as p1:
                wpool = p1.enter_context(tc.tile_pool(name="w1", bufs=1))
                act = p1.enter_context(tc.tile_pool(name="act1", bufs=1))
                dbl = p1.enter_context(tc.tile_pool(name="dbl1", bufs=2))
                scratch = p1.enter_context(tc.tile_pool(name="scr1", bufs=1))
                ps_ev = p1.enter_context(tc.tile_pool(name="ps_ev", bufs=4, space="PSUM"))
                ps_stat = p1.enter_context(tc.tile_pool(name="ps_st", bufs=1, space="PSUM"))
                ps_bc = p1.enter_context(tc.tile_pool(name="ps_bc", bufs=1, space="PSUM"))

                wk_t = wpool.tile([128, CB, C], f32r, tag="wk")
                dma8(wk_t, Wk.rearrange("(a p) m -> p a m", p=128))
                wv_t = wpool.tile([128, CB, C], f32r, tag="wv")
                dma8(wv_t, Wv.rearrange("(a p) m -> p a m", p=128))
                wr_t = wpool.tile([128, CB, C], f32r, tag="wr")
                dma8(wr_t, Wr.rearrange("(a p) m -> p a m", p=128))

                for ic in range(NCH):
                    t0 = ic * TC
                    x_t = dbl.tile([128, CB, TC], f32r, tag="x")
                    dma8(x_t, xT.rearrange("(cb p) t -> p cb t", p=128)[:, :, t0:t0 + TC], parts=4)
                    h_t = dbl.tile([128, CB, TC + 1], f32, tag="h")
                    nc.vector.tensor_copy(out=h_t[:, :, 0:1], in_=carryH)
                    layernorm((dbl, ps_stat, ps_bc, scratch, "d", "d"), x_t,
                              lambda cb: cv[:, cb, LN1G:LN1G + 1],
                              lambda cb: cv[:, cb, LN1B:LN1B + 1], h_t)
                    nc.vector.tensor_copy(out=carryH, in_=h_t[:, :, TC:TC + 1])

                    d_t = dbl.tile([128, CB, TC], f32, tag="d")
                    nc.vector.tensor_sub(d_t, h_t[:, :, 1:TC + 1], h_t[:, :, 0:TC])

                    ek = dbl.tile([128, CB, TC], f32, tag="ek")
                    ekv = dbl.tile([128, CB, TC], f32, tag="ekv")
                    rsig = dbl.tile([128, CB, TC], bf16, tag="rsig")

                    for which, w_t, tmrow in (("k", wk_t, TMK), ("v", wv_t, TMV),
                                              ("r", wr_t, TMR)):
                        in_t = dbl.tile([128, CB, TC], f32r, tag="min")
                        for cb in range(CB):
                            nc.vector.scalar_tensor_tensor(
                                out=in_t[:, cb, :], in0=d_t[:, cb, :],
                                scalar=cv[:, cb, tmrow:tmrow + 1],
                                in1=h_t[:, cb, 0:TC], op0=OP.mult, op1=OP.add)
                        for co in range(CB):
                            ps = ps_ev.tile([128, TC], f32, tag="ev")
                            csl = slice(co * 128, (co + 1) * 128)
                            for a in range(CB):
                                nc.tensor.matmul(ps, w_t[:, a, csl], in_t[:, a, :],
                                                 start=(a == 0), stop=(a == CB - 1))
                            if which == "k":
                                nc.scalar.activation(out=ek[:, co, :], in_=ps, func=AF.Exp)
                            elif which == "v":
                                nc.vector.tensor_mul(ekv[:, co, :], ek[:, co, :], ps)
                            else:
                                nc.scalar.activation(out=rsig[:, co, :], in_=ps,
                                                     func=AF.Sigmoid)
                    dma8_out(ekd[ic], ek, 4)
                    dma8_out(ekvd[ic], ekv, 4)
                    dma8_out(rsd1[ic], rsig, 2)

              # ================= Phase 1b: WKV scan + Wo + residual ============
              with contextlib.ExitStack() as p1b:
                wpool = p1b.enter_context(tc.tile_pool(name="w1b", bufs=1))
                act = p1b.enter_context(tc.tile_pool(name="act1b", bufs=1))
                dbl = p1b.enter_context(tc.tile_pool(name="dbl1b", bufs=2))
                ps_ev = p1b.enter_context(tc.tile_pool(name="ps_evb", bufs=4, space="PSUM"))

                wo_t = wpool.tile([128, CB, C], bf16, tag="wo")
                dma8(wo_t, Wo.rearrange("(a p) m -> p a m", p=128))

                for ic in range(NCH):
                    t0 = ic * TC
                    ek = dbl.tile([128, CB, TC], f32, tag="ekb")
                    dma8(ek, ekd[ic], parts=4)
                    ekv = dbl.tile([128, CB, TC], f32, tag="ekvb")
                    dma8(ekv, ekvd[ic], parts=4)
                    rsig = dbl.tile([128, CB, TC], bf16, tag="rsigb")
                    dma8(rsig, rsd1[ic], parts=2)
                    x_t = dbl.tile([128, CB, TC], f32, tag="xb")
                    dma8(x_t, xT.bitcast(f32).rearrange("(cb p) t -> p cb t", p=128)[:, :, t0:t0 + TC], parts=4)

                    A_t = dbl.tile([128, CB, TC + 1], f32, tag="A")
                    B_t = dbl.tile([128, CB, TC + 1], f32, tag="B")
                    nc.vector.tensor_copy(out=A_t[:, :, 0:1], in_=carryA)
                    nc.vector.tensor_copy(out=B_t[:, :, 0:1], in_=carryB)
                    for cb in range(CB):
                        ew_b = _bcast_free(cv[:, cb, EW:EW + 1], TC)
                        nc.vector.tensor_tensor_scan(
                            out=A_t[:, cb, 1:TC + 1], data0=ew_b, data1=ekv[:, cb, :],
                            initial=A_t[:, cb, 0:1], op0=OP.mult, op1=OP.add)
                        nc.vector.tensor_tensor_scan(
                            out=B_t[:, cb, 1:TC + 1], data0=ew_b, data1=ek[:, cb, :],
                            initial=B_t[:, cb, 0:1], op0=OP.mult, op1=OP.add)
                    nc.vector.tensor_copy(out=carryA, in_=A_t[:, :, TC:TC + 1])
                    nc.vector.tensor_copy(out=carryB, in_=B_t[:, :, TC:TC + 1])

                    # num -> ekv, den -> ek (in place)
                    for cb in range(CB):
                        eu_s = cv[:, cb, EU:EU + 1]
                        nc.vector.scalar_tensor_tensor(
                            out=ekv[:, cb, :], in0=ekv[:, cb, :], scalar=eu_s,
                            in1=A_t[:, cb, 0:TC], op0=OP.mult, op1=OP.add)
                        nc.vector.scalar_tensor_tensor(
                            out=ek[:, cb, :], in0=ek[:, cb, :], scalar=eu_s,
                            in1=B_t[:, cb, 0:TC], op0=OP.mult, op1=OP.add)
                    nc.vector.reciprocal_approx_fast(out=ek, in_=ek)
                    nc.gpsimd.tensor_mul(ekv, ekv, ek)          # wkv
                    y_t = dbl.tile([128, CB, TC], bf16, tag="yb")
                    nc.vector.tensor_mul(y_t, ekv, rsig)        # r_sig * wkv

                    x2_t = dbl.tile([128, CB, TC], f32, tag="x2a")
                    for co in range(CB):
                        ps = ps_ev.tile([128, TC], f32, tag="ev")
                        csl = slice(co * 128, (co + 1) * 128)
                        for a in range(CB):
                            nc.tensor.matmul(ps, wo_t[:, a, csl], y_t[:, a, :],
                                             start=(a == 0), stop=(a == CB - 1))
                        nc.vector.tensor_add(x2_t[:, co, :], x_t[:, co, :], ps)
                    dma8_out(x2d[ic], x2_t, 4)

            # ================= Phase 2a: FFN kk/rr production =================
            if "2" in _PH:
              kkd = dramp.tile([NCH, 128, FB, TC], bf16, tag="kkd")
              rsd = dramp.tile([NCH, 128, CB, TC], bf16, tag="rsd")
              with contextlib.ExitStack() as p2:
                wpool = p2.enter_context(tc.tile_pool(name="w2", bufs=1))
                act = p2.enter_context(tc.tile_pool(name="act2", bufs=1))
                dbl = p2.enter_context(tc.tile_pool(name="dbl2", bufs=2))
                scratch = p2.enter_context(tc.tile_pool(name="scr2", bufs=1))
                rel = p2.enter_context(tc.tile_pool(name="rel", bufs=2))
                ps_ev = p2.enter_context(tc.tile_pool(name="ps_ev2", bufs=4, space="PSUM"))
                ps_stat = p2.enter_context(tc.tile_pool(name="ps_st2", bufs=1, space="PSUM"))
                ps_bc = p2.enter_context(tc.tile_pool(name="ps_bc2", bufs=1, space="PSUM"))

                fwk_t = wpool.tile([128, CB, 4 * C], bf16, tag="fwk")
                dma8(fwk_t, fWk.rearrange("(a p) m -> p a m", p=128))
                fwr_t = wpool.tile([128, CB, C], bf16, tag="fwr")
                dma8(fwr_t, fWr.rearrange("(a p) m -> p a m", p=128))

                for ic in range(NCH):
                    x_t = dbl.tile([128, CB, TC], f32r, tag="x2")
                    dma8(x_t, x2d[ic].bitcast(f32r), parts=4)
                    g_t = dbl.tile([128, CB, TC + 1], f32, tag="g")
                    nc.vector.tensor_copy(out=g_t[:, :, 0:1], in_=carryG)
                    layernorm((dbl, ps_stat, ps_bc, scratch, "d2", "d2"), x_t,
                              lambda cb: cv[:, cb, LN2G:LN2G + 1],
                              lambda cb: cv[:, cb, LN2B:LN2B + 1], g_t)
                    nc.vector.tensor_copy(out=carryG, in_=g_t[:, :, TC:TC + 1])

                    d_t = dbl.tile([128, CB, TC], f32, tag="d2")
                    nc.vector.tensor_sub(d_t, g_t[:, :, 1:TC + 1], g_t[:, :, 0:TC])
                    fin_k = dbl.tile([128, CB, TC], bf16, tag="fink")
                    fin_r = dbl.tile([128, CB, TC], bf16, tag="finr")
                    for cb in range(CB):
                        nc.vector.scalar_tensor_tensor(
                            out=fin_k[:, cb, :], in0=d_t[:, cb, :],
                            scalar=cv[:, cb, FTMK:FTMK + 1],
                            in1=g_t[:, cb, 0:TC], op0=OP.mult, op1=OP.add)
                        nc.vector.scalar_tensor_tensor(
                            out=fin_r[:, cb, :], in0=d_t[:, cb, :],
                            scalar=cv[:, cb, FTMR:FTMR + 1],
                            in1=g_t[:, cb, 0:TC], op0=OP.mult, op1=OP.add)

                    kk = dbl.tile([128, FB, TC], bf16, tag="kk")
                    for co in range(FB):
                        ps = ps_ev.tile([128, TC], f32, tag="ev2")
                        csl = slice(co * 128, (co + 1) * 128)
                        for a in range(CB):
                            nc.tensor.matmul(ps, fwk_t[:, a, csl], fin_k[:, a, :],
                                             start=(a == 0), stop=(a == CB - 1))
                        rt = rel.tile([128, TC], f32, tag="rt")
                        nc.scalar.activation(out=rt, in_=ps, func=AF.Relu)
                        nc.vector.tensor_mul(kk[:, co, :], rt, rt)
                    dma8_out(kkd[ic], kk, 8)

                    rsig2 = dbl.tile([128, CB, TC], bf16, tag="rsig2")
                    for co in range(CB):
                        ps = ps_ev.tile([128, TC], f32, tag="ev2")
                        csl = slice(co * 128, (co + 1) * 128)
                        for a in range(CB):
                            nc.tensor.matmul(ps, fwr_t[:, a, csl], fin_r[:, a, :],
                                             start=(a == 0), stop=(a == CB - 1))
                        nc.scalar.activation(out=rsig2[:, co, :], in_=ps, func=AF.Sigmoid)
                    dma8_out(rsd[ic], rsig2, 2)

              # ================= Phase 2b: FFN down-proj + residual ============
              with contextlib.ExitStack() as p2b:
                wpool = p2b.enter_context(tc.tile_pool(name="w2b", bufs=1))
                dbl = p2b.enter_context(tc.tile_pool(name="dbl2b", bufs=2))
                ps_ffn = p2b.enter_context(tc.tile_pool(name="ps_ffn", bufs=4, space="PSUM"))

                fwv_t = wpool.tile([128, FB, C], bf16, tag="fwv")
                dma8(fwv_t, fWv.rearrange("(a p) m -> p a m", p=128), parts=16)

                for ic in range(NCH):
                    t0 = ic * TC
                    kk = dbl.tile([128, FB, TC], bf16, tag="kkb")
                    dma8(kk, kkd[ic], parts=8)
                    rsig2 = dbl.tile([128, CB, TC], bf16, tag="rsig2b")
                    dma8(rsig2, rsd[ic], parts=2)
                    x2_t = dbl.tile([128, CB, TC], f32, tag="x2b")
                    dma8(x2_t, x2d[ic], parts=4)
                    out_t = dbl.tile([128, CB, TC], f32, tag="outb")
                    for co in range(CB):
                        ps = ps_ffn.tile([128, TC], f32, tag="ffn")
                        csl = slice(co * 128, (co + 1) * 128)
                        for a in range(FB):
                            nc.tensor.matmul(ps, fwv_t[:, a, csl], kk[:, a, :],
                                             start=(a == 0), stop=(a == FB - 1))
                        nc.vector.tensor_mul(ps, rsig2[:, co, :], ps)
                        nc.vector.tensor_add(out_t[:, co, :], x2_t[:, co, :], ps)
                    dma8_out(outT.rearrange("(cb p) t -> p cb t", p=128)[:, :, t0:t0 + TC], out_t, 4)

    nc.finalize()
    return nc


def _prep_maps(inputs):
    x = np.asarray(inputs["x"], np.float32)
    ew = np.exp(-np.exp(np.asarray(inputs["time_decay"], np.float32))).astype(np.float32)
    eu = np.exp(np.asarray(inputs["time_first"], np.float32)).astype(np.float32)
    cvecs = np.stack([
        np.asarray(inputs["ln1_g"], np.float32), np.asarray(inputs["ln1_b"], np.float32),
        np.asarray(inputs["ln2_g"], np.float32), np.asarray(inputs["ln2_b"], np.float32),
        np.asarray(inputs["tmk"], np.float32), np.asarray(inputs["tmv"], np.float32),
        np.asarray(inputs["tmr"], np.float32), np.asarray(inputs["ftmk"], np.float32),
        np.asarray(inputs["ftmr"], np.float32), ew, eu,
        np.zeros(C, np.float32),
    ]).astype(np.float32)
    cvecs = np.ascontiguousarray(cvecs.reshape(12, CB, 128).transpose(2, 1, 0))
    common = {
        "cvecs": cvecs,
        "ones128": np.ones(128, np.float32),
        "Wk": np.asarray(inputs["Wk"], np.float32),
        "Wv": np.asarray(inputs["Wv"], np.float32),
        "Wr": np.asarray(inputs["Wr"], np.float32),
        "Wo": np.asarray(inputs["Wo"]).astype(ml_dtypes.bfloat16),
        "fWk": np.asarray(inputs["fWk"]).astype(ml_dtypes.bfloat16),
        "fWv": np.asarray(inputs["fWv"]).astype(ml_dtypes.bfloat16),
        "fWr": np.asarray(inputs["fWr"]).astype(ml_dtypes.bfloat16),
    }
    return [{**common, "xT": np.ascontiguousarray(x[b].T)} for b in range(B)]


def get_nc():
    if "nc" not in _CACHE:
        _CACHE["nc"] = _build()
    return _CACHE["nc"]


def kernel(**inputs):
    from concourse.bass_utils import run_bass_kernel_spmd
    nc = get_nc()
    in_maps = _prep_maps(inputs)
    res = run_bass_kernel_spmd(nc, in_maps, core_ids=list(range(B)))
    return np.stack([np.ascontiguousarray(r["outT"].T) for r in res.results])



# revision 8
# speedup vs baseline: 1.1662x; 1.1662x over previous
"""RWKV v4 block (nn_Block_15109694947416) on 8 TRN2 NeuronCores.

Strategy (v2):
- Data-parallel over B: core i processes batch i (B=8). No collectives.
- Channel-major [C, T] on-chip layout, T in 4 chunks of 512.
- LayerNorm gain/bias and the token-shift mixing (x*tm + shift(x)*(1-tm)) are
  folded into the weights: k = u8 @ (g*tm*Wk) + shift(u8) @ (g*(1-tm)*Wk)
  + const, where u = 16*(x-m)*rstd is the fp8-quantized normalized input and
  the shifted operand is the SAME tile at a one-column offset, consumed by a
  single DoubleRow fp8 matmul per (k-block, out-block). Constants ride the
  activation bias; 1/scales ride the activation scale (per-partition APs).
- fp8e4 (e4m3) + MatmulPerfMode.DoubleRow for Wk/Wv/Wr (folded pairs), Wo and
  fWr; bf16 for fWk/fWv (precision headroom).
- rstd = exp(-0.5*ln(var+eps)) and sigmoid via exp:
  sigmoid(q)*z = z / (den*(1+exp(-q))) so the whole kernel uses only the
  natural_log_exp activation table (no table swaps).
- WKV scan unstabilized in fp32 scan-state (exact for this regime), carried
  bf16 between chunks; elementwise in bf16 where precision allows (DVE
  2x/4x modes).
- Residual path (x, x2, out) stays fp32 end to end.
"""

import math
import numpy as np
import ml_dtypes

B, T, C = 8, 2048, 1024
TC = 512                 # time chunk
NCH = T // TC            # chunks (4)
CB = C // 128            # channel blocks (8)
FB = 4 * C // 128        # ffn hidden blocks (32)
EPS = 1e-5
SU = 16.0                # u-activation scale (u8 stores 16*u)
SV = 32.0                # v/y chain scale (t_v stores 32*v, y8 stores 32*y)
NROW = 16
CVW = 200                # cvall [128, 200]: 8*16 tm rows, 32*2 ffn, ones

_CACHE = {}

# per-(cb) const rows: cvall col = cb*NROW + row
(EW, EU, CK, CV32, CRN, U1INIT, U2INIT, FTMK, FTMR,
 SCK, SCV, SCRN, SCO, SCFR, EPSR, LNSCR) = range(NROW)
# per-(ffn co) rows: col = 128 + co*2 + row
CFK, CFRN = range(2)
ONES_COL = 192           # 1.0 f32 (bitcast f32r for ones-matmul lhsT)
# carries tile [128, CB, 4] bf16 rows
CAR_U, CAR_U2, CAR_A, CAR_B = range(4)


def _bcast_free(ap, n):
    """[128,1] AP -> [128,n] stride-0 broadcast along free dim."""
    import concourse.bass as bass
    return bass.AP(tensor=ap.tensor, offset=ap.offset, ap=[ap.ap[0], [0, n]])


def _bcast_mid(ap, nmid):
    """[128,N] AP -> [128,nmid,N] stride-0 broadcast of a middle dim."""
    import concourse.bass as bass
    return bass.AP(tensor=ap.tensor, offset=ap.offset,
                   ap=[ap.ap[0], [0, nmid], ap.ap[1]])


def _pair_shift(t, a, n):
    """u-tile [128, CB, n+1] -> [128, 2, n] AP at block a: [p, i, t] =
    u[p, a, i + t]  (i=0: shifted/prev token, i=1: current token)."""
    import concourse.bass as bass
    ap = t[:, a, :]
    return bass.AP(tensor=ap.tensor, offset=ap.offset,
                   ap=[ap.ap[0], [1, 2], [1, n]])


def _build():
    import concourse.bass as bass
    import concourse.bacc as bacc
    import concourse.tile as tile
    import contextlib
    from concourse import mybir

    f32 = mybir.dt.float32
    f32r = mybir.dt.float32r
    bf16 = mybir.dt.bfloat16
    fp8 = mybir.dt.float8e4
    AF = mybir.ActivationFunctionType
    OP = mybir.AluOpType
    DR = mybir.MatmulPerfMode.DoubleRow

    nc = bacc.Bacc(None, target_bir_lowering=False, debug=False)

    xT = nc.dram_tensor("xT", [C, T], f32r, kind="ExternalInput")
    cvd = nc.dram_tensor("cvall", [128, CVW], f32r, kind="ExternalInput")
    ones16_in = nc.dram_tensor("ones128b", [128], bf16, kind="ExternalInput")
    ones_bin = nc.dram_tensor("onesb", [128], f32r, kind="ExternalInput")
    Wk2 = nc.dram_tensor("Wk2", [C, 2, C], fp8, kind="ExternalInput")
    Wv2 = nc.dram_tensor("Wv2", [C, 2, C], fp8, kind="ExternalInput")
    Wr2 = nc.dram_tensor("Wr2", [C, 2, C], fp8, kind="ExternalInput")
    Wo8 = nc.dram_tensor("Wo8", [C, C], fp8, kind="ExternalInput")
    fWk16 = nc.dram_tensor("fWk16", [C, 4 * C], bf16, kind="ExternalInput")
    fWr8 = nc.dram_tensor("fWr8", [C, C], fp8, kind="ExternalInput")
    fWv16 = nc.dram_tensor("fWv16", [4 * C, C], bf16, kind="ExternalInput")
    outT = nc.dram_tensor("outT", [C, T], f32, kind="ExternalOutput")

    xTr = xT.rearrange("(cb p) t -> p cb t", p=128)
    outTr = outT.rearrange("(cb p) t -> p cb t", p=128)

    with tile.TileContext(nc) as tc:
      with contextlib.ExitStack() as ctx:
        consts = ctx.enter_context(tc.tile_pool(name="consts", bufs=1))
        dramp = ctx.enter_context(tc.tile_pool(name="dram", bufs=1, space="DRAM"))

        cvt = consts.tile([128, CVW], f32r)
        nc.sync.dma_start(out=cvt, in_=cvd[:, :])
        cvtf = cvt.bitcast(f32)
        ones_b = consts.tile([1, 128], f32r)
        nc.sync.dma_start(out=ones_b, in_=ones_bin.rearrange("(o p) -> o p", o=1))
        ones_k16 = consts.tile([128, 1], bf16)
        nc.sync.dma_start(out=ones_k16, in_=ones16_in.rearrange("(p o) -> p o", o=1))
        ones_k = cvt[:, ONES_COL:ONES_COL + 1]

        def cva(cb, row):
            i = cb * NROW + row
            return cvtf[:, i:i + 1]

        def cvf(co, row):
            i = 128 + co * 2 + row
            return cvtf[:, i:i + 1]

        car = consts.tile([128, CB, 4], bf16)
        nc.vector.tensor_copy(out=car[:, :, CAR_U:CAR_U + 1],
                              in_=cvtf[:, 0:128].rearrange(
                                  "p (cb r) -> p cb r", r=NROW)[:, :, U1INIT:U1INIT + 1])
        nc.vector.tensor_copy(out=car[:, :, CAR_U2:CAR_U2 + 1],
                              in_=cvtf[:, 0:128].rearrange(
                                  "p (cb r) -> p cb r", r=NROW)[:, :, U2INIT:U2INIT + 1])
        nc.vector.memset(car[:, :, CAR_A:CAR_B + 1], 0.0)

        x2d = dramp.tile([NCH, 128, CB, TC], f32)
        kkd = dramp.tile([NCH, 128, FB, TC], bf16, tag="kkd")
        rrd = dramp.tile([NCH, 128, CB, TC], bf16, tag="rrd")

        def layernorm_stats(pools, x_t, sq16, rows, tmp):
            """Per-token mean + 16*rstd rows from x_t [128,CB,TC] f32.

            rows[:,0,:]=m  rows[:,1,:]=16*rstd (f32r-typed for the broadcast
            matmuls); rstd = exp(-0.5*ln(var+eps)+ln(16)) stays on the exp/ln
            activation table.
            """
            ps_stx, ps_stq = pools
            nc.scalar.activation(out=sq16, in_=x_t.bitcast(f32), func=AF.Square)
            for cb in range(CB):
                nc.tensor.matmul(ps_stx, ones_k, x_t[:, cb, :],
                                 start=(cb == 0), stop=(cb == CB - 1))
            for cb in range(CB):
                nc.tensor.matmul(ps_stq, ones_k16, sq16[:, cb, :],
                                 start=(cb == 0), stop=(cb == CB - 1))
            rowf = rows.bitcast(f32)
            nc.vector.tensor_scalar_mul(rows[:, 0, :], ps_stx, 1.0 / C)
            nc.vector.tensor_mul(tmp[:, 0, :], rowf[:, 0, :], rowf[:, 0, :])
            nc.vector.scalar_tensor_tensor(
                out=tmp[:, 1, :], in0=ps_stq, scalar=1.0 / C,
                in1=tmp[:, 0, :], op0=OP.mult, op1=OP.subtract)
            nc.scalar.activation(out=tmp[:, 0, :], in_=tmp[:, 1, :],
                                 func=AF.Ln, bias=cvtf[0:1, EPSR + 0:EPSR + 1])
            nc.scalar.activation(out=tmp[:, 1, :], in_=tmp[:, 0, :],
                                 func=AF.Exp, scale=-0.5,
                                 bias=cvtf[0:1, LNSCR:LNSCR + 1])
            nc.vector.tensor_copy(out=rows[:, 1, :], in_=tmp[:, 1, :])

        # ================= Phase 1: fused time-mix =================
        with contextlib.ExitStack() as p1:
            wpool = p1.enter_context(tc.tile_pool(name="w1", bufs=1))
            dbl = p1.enter_context(tc.tile_pool(name="dbl1", bufs=2))
            sgl = p1.enter_context(tc.tile_pool(name="sgl1", bufs=1))
            rowp = p1.enter_context(tc.tile_pool(name="rows1", bufs=1))
            ps_mm = p1.enter_context(tc.tile_pool(name="ps_mm", bufs=3, space="PSUM"))
            ps_st = p1.enter_context(tc.tile_pool(name="ps_st", bufs=1, space="PSUM"))
            ps_bc = p1.enter_context(tc.tile_pool(name="ps_bc", bufs=1, space="PSUM"))

            wk_t = wpool.tile([128, CB, 2, C], fp8, tag="wk")
            nc.sync.dma_start(out=wk_t, in_=Wk2.rearrange("(a p) i m -> p a i m", p=128))
            wv_t = wpool.tile([128, CB, 2, C], fp8, tag="wv")
            nc.sync.dma_start(out=wv_t, in_=Wv2.rearrange("(a p) i m -> p a i m", p=128))
            wr_t = wpool.tile([128, CB, 2, C], fp8, tag="wr")
            nc.sync.dma_start(out=wr_t, in_=Wr2.rearrange("(a p) i m -> p a i m", p=128))
            wo_t = wpool.tile([128, CB, C], fp8, tag="wo")
            nc.sync.dma_start(out=wo_t, in_=Wo8.rearrange("(a p) m -> p a m", p=128))

            for ic in range(NCH):
                t0 = ic * TC
                x_t = dbl.tile([128, CB, TC], f32r, tag="x")
                nc.sync.dma_start(out=x_t, in_=xTr[:, :, t0:t0 + TC])
                x_f = x_t.bitcast(f32)

                sq16 = sgl.tile([128, CB, TC], bf16, tag="sq")
                rows = rowp.tile([1, 2, TC], f32r, tag="rows")
                tmp = rowp.tile([1, 2, TC], f32, tag="rtmp")
                ps_stx = ps_st.tile([1, TC], f32, tag="stx")
                ps_stq = ps_st.tile([1, TC], f32, tag="stq")
                layernorm_stats((ps_stx, ps_stq), x_t, sq16, rows, tmp)
                bc = ps_bc.tile([128, 2, TC], f32, tag="bc")
                nc.tensor.matmul(bc[:, 0, :], ones_b, rows[:, 0, :])
                nc.tensor.matmul(bc[:, 1, :], ones_b, rows[:, 1, :])

                s1 = sgl.tile([128, CB, TC], f32, tag="s1")
                nc.vector.tensor_sub(s1, x_f, _bcast_mid(bc[:, 0, :], CB))
                # u8p[:, cb, 1, :] = current token (16*u); [:, cb, 0, :] = prev
                u8p = dbl.tile([128, CB, 2, TC], fp8, tag="u8")
                nc.vector.tensor_mul(u8p[:, :, 1, :], s1,
                                     _bcast_mid(bc[:, 1, :], CB))
                nc.vector.tensor_copy(out=u8p[:, :, 0, 0:1],
                                      in_=car[:, :, CAR_U:CAR_U + 1])
                nc.vector.tensor_copy(out=u8p[:, :, 0, 1:TC],
                                      in_=u8p[:, :, 1, 0:TC - 1])
                nc.vector.tensor_copy(out=car[:, :, CAR_U:CAR_U + 1],
                                      in_=u8p[:, :, 1, TC - 1:TC])

                ek = sgl.tile([128, CB, TC], bf16, tag="ek")
                ekv = sgl.tile([128, CB, TC], bf16, tag="ekv")
                e_r = sgl.tile([128, CB, TC], bf16, tag="er")
                for wt, post in (
                    (wk_t, lambda co, ps: nc.scalar.activation(
                        out=ek[:, co, :], in_=ps, func=AF.Exp,
                        scale=cva(0, SCK), bias=cva(co, CK))),
                    (wv_t, lambda co, ps: nc.scalar.activation(
                        out=ekv[:, co, :], in_=ps, func=AF.Identity,
                        scale=cva(0, SCV), bias=cva(co, CV32))),
                    (wr_t, lambda co, ps: nc.scalar.activation(
                        out=e_r[:, co, :], in_=ps, func=AF.Exp,
                        scale=cva(0, SCRN), bias=cva(co, CRN))),
                ):
                    for co in range(CB):
                        ps = ps_mm.tile([128, TC], f32, tag="mm")
                        csl = slice(co * 128, (co + 1) * 128)
                        for a in range(CB):
                            nc.tensor.matmul(ps, wt[:, a, :, csl],
                                             u8p[:, a, :, :],
                                             start=(a == 0), stop=(a == CB - 1),
                                             perf_mode=DR)
                        post(co, ps)

                # ekv = (32*v) * exp(k)   (in place)
                nc.vector.tensor_mul(ekv, ekv, ek)

                AB = sgl.tile([128, 2, CB, TC + 1], bf16, tag="AB")
                nc.vector.tensor_copy(out=AB[:, 0, :, 0:1], in_=car[:, :, CAR_A:CAR_A + 1])
                nc.vector.tensor_copy(out=AB[:, 1, :, 0:1], in_=car[:, :, CAR_B:CAR_B + 1])
                for cb in range(CB):
                    ew_b = _bcast_free(cva(cb, EW), TC)
                    nc.vector.tensor_tensor_scan(
                        out=AB[:, 0, cb, 1:TC + 1], data0=ew_b, data1=ekv[:, cb, :],
                        initial=AB[:, 0, cb, 0:1], op0=OP.mult, op1=OP.add)
                    nc.vector.tensor_tensor_scan(
                        out=AB[:, 1, cb, 1:TC + 1], data0=ew_b, data1=ek[:, cb, :],
                        initial=AB[:, 1, cb, 0:1], op0=OP.mult, op1=OP.add)
                nc.vector.tensor_copy(out=car[:, :, CAR_A:CAR_A + 1],
                                      in_=AB[:, 0, :, TC:TC + 1])
                nc.vector.tensor_copy(out=car[:, :, CAR_B:CAR_B + 1],
                                      in_=AB[:, 1, :, TC:TC + 1])

                # num -> ekv, den -> ek (in place)
                for cb in range(CB):
                    eu_s = cva(cb, EU)
                    nc.vector.scalar_tensor_tensor(
                        out=ekv[:, cb, :], in0=ekv[:, cb, :], scalar=eu_s,
                        in1=AB[:, 0, cb, 0:TC], op0=OP.mult, op1=OP.add)
                    nc.vector.scalar_tensor_tensor(
                        out=ek[:, cb, :], in0=ek[:, cb, :], scalar=eu_s,
                        in1=AB[:, 1, cb, 0:TC], op0=OP.mult, op1=OP.add)
                # den2 = den * (1 + e_r): folds the r-sigmoid into the division
                den2 = sgl.tile([128, CB, TC], f32, tag="s1")
                nc.vector.scalar_tensor_tensor(
                    out=den2, in0=e_r, scalar=1.0, in1=ek,
                    op0=OP.add, op1=OP.mult)
                nc.vector.reciprocal_approx_fast(out=den2, in_=den2)
                y8 = sgl.tile([128, CB, TC], fp8, tag="er")
                nc.vector.tensor_mul(y8, ekv, den2)

                x2 = sgl.tile([128, CB, TC], f32, tag="x2")
                for co in range(CB):
                    ps = ps_mm.tile([128, TC], f32, tag="mm")
                    csl = slice(co * 128, (co + 1) * 128)
                    for j in range(CB // 2):
                        nc.tensor.matmul(ps, wo_t[:, 2 * j:2 * j + 2, csl],
                                         y8[:, 2 * j:2 * j + 2, :],
                                         start=(j == 0), stop=(j == CB // 2 - 1),
                                         perf_mode=DR)
                    nc.vector.scalar_tensor_tensor(
                        out=x2[:, co, :], in0=ps, scalar=cva(0, SCO),
                        in1=x_f[:, co, :], op0=OP.mult, op1=OP.add)
                nc.sync.dma_start(out=x2d[ic], in_=x2)

        # ================= Phase 2a: FFN up (kk + rr) =================
        with contextlib.ExitStack() as p2:
            wpool = p2.enter_context(tc.tile_pool(name="w2", bufs=1))
            dbl = p2.enter_context(tc.tile_pool(name="dbl2", bufs=2))
            sgl = p2.enter_context(tc.tile_pool(name="sgl2", bufs=1))
            rowp = p2.enter_context(tc.tile_pool(name="rows2", bufs=1))
            ps_mm = p2.enter_context(tc.tile_pool(name="ps_mm2", bufs=3, space="PSUM"))
            ps_st = p2.enter_context(tc.tile_pool(name="ps_st2", bufs=1, space="PSUM"))
            ps_bc = p2.enter_context(tc.tile_pool(name="ps_bc2", bufs=1, space="PSUM"))

            fwk_t = wpool.tile([128, CB, 4 * C], bf16, tag="fwk")
            nc.sync.dma_start(out=fwk_t, in_=fWk16.rearrange("(a p) m -> p a m", p=128))
            fwr_t = wpool.tile([128, CB, C], fp8, tag="fwr")
            nc.sync.dma_start(out=fwr_t, in_=fWr8.rearrange("(a p) m -> p a m", p=128))

            for ic in range(NCH):
                x2_t = dbl.tile([128, CB, TC], f32r, tag="x2i")
                nc.sync.dma_start(out=x2_t, in_=x2d[ic].bitcast(f32r))
                x2_f = x2_t.bitcast(f32)

                sq16 = sgl.tile([128, CB, TC], bf16, tag="sq2")
                rows = rowp.tile([1, 2, TC], f32r, tag="rows")
                tmp = rowp.tile([1, 2, TC], f32, tag="rtmp")
                ps_stx = ps_st.tile([1, TC], f32, tag="stx")
                ps_stq = ps_st.tile([1, TC], f32, tag="stq")
                layernorm_stats((ps_stx, ps_stq), x2_t, sq16, rows, tmp)
                bc = ps_bc.tile([128, 2, TC], f32, tag="bc")
                nc.tensor.matmul(bc[:, 0, :], ones_b, rows[:, 0, :])
                nc.tensor.matmul(bc[:, 1, :], ones_b, rows[:, 1, :])

                s1 = sgl.tile([128, CB, TC], f32, tag="s12")
                nc.vector.tensor_sub(s1, x2_f, _bcast_mid(bc[:, 0, :], CB))
                u2 = sgl.tile([128, CB, TC + 1], bf16, tag="u2")
                nc.vector.tensor_copy(out=u2[:, :, 0:1], in_=car[:, :, CAR_U2:CAR_U2 + 1])
                nc.vector.tensor_mul(u2[:, :, 1:TC + 1], s1,
                                     _bcast_mid(bc[:, 1, :], CB))
                nc.vector.tensor_copy(out=car[:, :, CAR_U2:CAR_U2 + 1],
                                      in_=u2[:, :, TC:TC + 1])

                dif = sgl.tile([128, CB, TC], bf16, tag="sq2")
                nc.vector.tensor_sub(dif, u2[:, :, 1:TC + 1], u2[:, :, 0:TC])
                w16 = sgl.tile([128, CB, TC], bf16, tag="w16")
                fin_r8 = sgl.tile([128, CB, TC], fp8, tag="finr")
                for cb in range(CB):
                    nc.vector.scalar_tensor_tensor(
                        out=w16[:, cb, :], in0=dif[:, cb, :],
                        scalar=cva(cb, FTMK),
                        in1=u2[:, cb, 0:TC], op0=OP.mult, op1=OP.add)
                    nc.vector.scalar_tensor_tensor(
                        out=fin_r8[:, cb, :], in0=dif[:, cb, :],
                        scalar=cva(cb, FTMR),
                        in1=u2[:, cb, 0:TC], op0=OP.mult, op1=OP.add)

                for q in range(4):
                    rt = sgl.tile([128, CB, TC], bf16, tag="rt")
                    for j in range(CB):
                        co = q * CB + j
                        ps = ps_mm.tile([128, TC], f32, tag="mm")
                        csl = slice(co * 128, (co + 1) * 128)
                        for a in range(CB):
                            nc.tensor.matmul(ps, fwk_t[:, a, csl], w16[:, a, :],
                                             start=(a == 0), stop=(a == CB - 1))
                        nc.scalar.activation(out=rt[:, j, :], in_=ps, func=AF.Relu,
                                             bias=cvf(co, CFK))
                    kkq = dbl.tile([128, CB, TC], bf16, tag="kkq")
                    nc.scalar.activation(out=kkq, in_=rt, func=AF.Square)
                    nc.sync.dma_start(out=kkd[ic][:, q * CB:(q + 1) * CB, :], in_=kkq)

                e_rr = sgl.tile([128, CB, TC], bf16, tag="rt")
                for co in range(CB):
                    ps = ps_mm.tile([128, TC], f32, tag="mm")
                    csl = slice(co * 128, (co + 1) * 128)
                    for j in range(CB // 2):
                        nc.tensor.matmul(ps, fwr_t[:, 2 * j:2 * j + 2, csl],
                                         fin_r8[:, 2 * j:2 * j + 2, :],
                                         start=(j == 0), stop=(j == CB // 2 - 1),
                                         perf_mode=DR)
                    nc.scalar.activation(out=e_rr[:, co, :], in_=ps, func=AF.Exp,
                                         scale=cva(0, SCFR),
                                         bias=cvf(co, CFRN))
                den = sgl.tile([128, CB, TC], f32, tag="s12")
                nc.vector.tensor_scalar_add(out=den, in0=e_rr, scalar1=1.0)
                nc.vector.reciprocal_approx_fast(out=den, in_=den)
                rr16 = sgl.tile([128, CB, TC], bf16, tag="rr")
                nc.vector.tensor_copy(out=rr16, in_=den)
                nc.sync.dma_start(out=rrd[ic], in_=rr16)

        # ================= Phase 2b: FFN down + residual =================
        with contextlib.ExitStack() as p3:
            wpool = p3.enter_context(tc.tile_pool(name="w3", bufs=1))
            dbl = p3.enter_context(tc.tile_pool(name="dbl3", bufs=2))
            halfp = p3.enter_context(tc.tile_pool(name="half3", bufs=2))
            ps_mm = p3.enter_context(tc.tile_pool(name="ps_mm3", bufs=4, space="PSUM"))

            fwv_t = wpool.tile([128, FB, C], bf16, tag="fwv")
            nc.sync.dma_start(out=fwv_t, in_=fWv16.rearrange("(a p) m -> p a m", p=128))

            for ic in range(NCH):
                t0 = ic * TC
                kk0 = halfp.tile([128, FB // 2, TC], bf16, tag="kkh")
                nc.sync.dma_start(out=kk0, in_=kkd[ic][:, 0:FB // 2, :])
                kk1 = halfp.tile([128, FB // 2, TC], bf16, tag="kkh")
                nc.sync.dma_start(out=kk1, in_=kkd[ic][:, FB // 2:FB, :])
                rr16 = dbl.tile([128, CB, TC], bf16, tag="rri")
                nc.sync.dma_start(out=rr16, in_=rrd[ic])
                x2_t = dbl.tile([128, CB, TC], f32, tag="x2b")
                nc.sync.dma_start(out=x2_t, in_=x2d[ic])

                out_t = dbl.tile([128, CB, TC], f32, tag="out")
                for co in range(CB):
                    ps = ps_mm.tile([128, TC], f32, tag="mm")
                    csl = slice(co * 128, (co + 1) * 128)
                    for a in range(FB):
                        kkh = kk0 if a < FB // 2 else kk1
                        nc.tensor.matmul(ps, fwv_t[:, a, csl],
                                         kkh[:, a % (FB // 2), :],
                                         start=(a == 0), stop=(a == FB - 1))
                    nc.vector.tensor_mul(ps, rr16[:, co, :], ps)
                    nc.vector.tensor_add(out_t[:, co, :], x2_t[:, co, :], ps)
                nc.sync.dma_start(out=outTr[:, :, t0:t0 + TC], in_=out_t)

    nc.finalize()
    return nc


def _prep_maps(inputs):
    E4 = ml_dtypes.float8_e4m3
    BF = ml_dtypes.bfloat16
    f32 = np.float32

    x = np.asarray(inputs["x"], f32)
    ln1_g = np.asarray(inputs["ln1_g"], f32)
    ln1_b = np.asarray(inputs["ln1_b"], f32)
    ln2_g = np.asarray(inputs["ln2_g"], f32)
    ln2_b = np.asarray(inputs["ln2_b"], f32)
    tmk = np.asarray(inputs["tmk"], f32)
    tmv = np.asarray(inputs["tmv"], f32)
    tmr = np.asarray(inputs["tmr"], f32)
    ftmk = np.asarray(inputs["ftmk"], f32)
    ftmr = np.asarray(inputs["ftmr"], f32)
    ew = np.exp(-np.exp(np.asarray(inputs["time_decay"], f32))).astype(f32)
    eu = np.exp(np.asarray(inputs["time_first"], f32)).astype(f32)

    def pow2s(m, target=224.0):
        m = float(m)
        if m <= 0:
            return 1.0
        return 2.0 ** math.floor(math.log2(target / m))

    def fold2(W, tm, g, b):
        """Pack [(1-tm)*g*W | tm*g*W] * S as [C, 2, Co] e4m3; const = b@W."""
        W = np.asarray(W, f32)
        Wa = ((1.0 - tm) * g)[:, None] * W
        Wb = (tm * g)[:, None] * W
        S = pow2s(max(np.abs(Wa).max(), np.abs(Wb).max()))
        P = np.stack([Wa, Wb], axis=1) * S
        return np.ascontiguousarray(P).astype(E4), S, (b @ W).astype(f32)

    Wk2, Sk, ck = fold2(inputs["Wk"], tmk, ln1_g, ln1_b)
    Wv2, Sv, cv_ = fold2(inputs["Wv"], tmv, ln1_g, ln1_b)
    Wr2, Sr, cr_ = fold2(inputs["Wr"], tmr, ln1_g, ln1_b)

    Wo = np.asarray(inputs["Wo"], f32)
    So = pow2s(np.abs(Wo).max())
    Wo8 = (Wo * So).astype(E4)

    fWk = np.asarray(inputs["fWk"], f32)
    fWk16 = (ln2_g[:, None] * fWk / SU).astype(BF)
    cfk = (ln2_b @ fWk).astype(f32)

    fWr = np.asarray(inputs["fWr"], f32)
    fWrg = ln2_g[:, None] * fWr
    Sfr = pow2s(np.abs(fWrg).max())
    fWr8 = (fWrg * Sfr).astype(E4)
    cfr = (ln2_b @ fWr).astype(f32)

    fWv16 = np.asarray(inputs["fWv"], f32).astype(BF)

    def plane(vec):
        # [C] indexed by channel -> [128, CB] (p, cb)
        return np.ascontiguousarray(np.asarray(vec, f32).reshape(CB, 128).T)

    with np.errstate(divide="ignore", invalid="ignore"):
        u1i = np.where(ln1_g != 0, -SU * ln1_b / ln1_g, 0.0)
        u2i = np.where(ln2_g != 0, -SU * ln2_b / ln2_g, 0.0)
    u1i = np.clip(np.nan_to_num(u1i), -400, 400)
    u2i = np.clip(np.nan_to_num(u2i), -400, 400)

    rows = np.zeros((128, CB, NROW), f32)
    rows[:, :, EW] = plane(ew)
    rows[:, :, EU] = plane(eu)
    rows[:, :, CK] = plane(ck)
    rows[:, :, CV32] = plane(SV * cv_)
    rows[:, :, CRN] = plane(-cr_)
    rows[:, :, U1INIT] = plane(u1i)
    rows[:, :, U2INIT] = plane(u2i)
    rows[:, :, FTMK] = plane(ftmk)
    rows[:, :, FTMR] = plane(ftmr)
    rows[:, :, SCK] = 1.0 / (SU * Sk)
    rows[:, :, SCV] = SV / (SU * Sv)
    rows[:, :, SCRN] = -1.0 / (SU * Sr)
    rows[:, :, SCO] = 1.0 / (SV * So)
    rows[:, :, SCFR] = -1.0 / (SU * Sfr)
    rows[:, :, EPSR] = EPS
    rows[:, :, LNSCR] = math.log(SU)

    cvall = np.zeros((128, CVW), f32)
    cvall[:, 0:128] = rows.reshape(128, 128)
    ffn = np.zeros((128, FB, 2), f32)
    ffn[:, :, CFK] = np.ascontiguousarray(cfk.reshape(FB, 128).T)
    ffn[:, 0:CB, CFRN] = np.ascontiguousarray((-cfr).reshape(CB, 128).T)
    cvall[:, 128:192] = ffn.reshape(128, 64)
    cvall[:, ONES_COL] = 1.0

    common = {
        "cvall": cvall,
        "ones128b": np.ones(128, BF), "onesb": np.ones(128, f32),
        "Wk2": Wk2, "Wv2": Wv2, "Wr2": Wr2, "Wo8": Wo8,
        "fWk16": fWk16, "fWr8": fWr8, "fWv16": fWv16,
    }
    return [{**common, "xT": np.ascontiguousarray(x[b].T)} for b in range(B)]


def get_nc():
    if "nc" not in _CACHE:
        _CACHE["nc"] = _build()
    return _CACHE["nc"]


def kernel(**inputs):
    from concourse.bass_utils import run_bass_kernel_spmd
    nc = get_nc()
    in_maps = _prep_maps(inputs)
    res = run_bass_kernel_spmd(nc, in_maps, core_ids=list(range(B)))
    return np.stack([np.ascontiguousarray(r["outT"].T) for r in res.results])


# revision 10
# speedup vs baseline: 1.3131x; 1.1259x over previous
"""RWKV v4 block (nn_Block_15109694947416) on 8 TRN2 NeuronCores.

Strategy (v2):
- Data-parallel over B: core i processes batch i (B=8). No collectives.
- Channel-major [C, T] on-chip layout, T in 4 chunks of 512.
- LayerNorm gain/bias and the token-shift mixing (x*tm + shift(x)*(1-tm)) are
  folded into the weights: k = u8 @ (g*tm*Wk) + shift(u8) @ (g*(1-tm)*Wk)
  + const, where u = 16*(x-m)*rstd is the fp8-quantized normalized input and
  the shifted operand is the SAME tile at a one-column offset, consumed by a
  single DoubleRow fp8 matmul per (k-block, out-block). Constants ride the
  activation bias; 1/scales ride the activation scale (per-partition APs).
- fp8e4 (e4m3) + MatmulPerfMode.DoubleRow for Wk/Wv/Wr (folded pairs), Wo and
  fWr; bf16 for fWk/fWv (precision headroom).
- rstd = exp(-0.5*ln(var+eps)) and sigmoid via exp:
  sigmoid(q)*z = z / (den*(1+exp(-q))) so the whole kernel uses only the
  natural_log_exp activation table (no table swaps).
- WKV scan unstabilized in fp32 scan-state (exact for this regime), carried
  bf16 between chunks; elementwise in bf16 where precision allows (DVE
  2x/4x modes).
- Residual path (x, x2, out) stays fp32 end to end.
"""

import math
import numpy as np
import ml_dtypes

B, T, C = 8, 2048, 1024
TC = 512                 # time chunk
NCH = T // TC            # chunks (4)
CB = C // 128            # channel blocks (8)
FB = 4 * C // 128        # ffn hidden blocks (32)
EPS = 1e-5
SU = 16.0                # u-activation scale (u8 stores 16*u)
SV = 32.0                # v/y chain scale (t_v stores 32*v, y8 stores 32*y)
NROW = 16
CVW = 200                # cvall [128, 200]: 8*16 tm rows, 32*2 ffn, ones

_CACHE = {}

# per-(cb) const rows: cvall col = cb*NROW + row
(EW, EU, CK, CV32, CRN, U1INIT, U2INIT, FTMK, FTMR,
 SCK, SCV, SCRN, SCO, SCFR, EPSR, LNSCR) = range(NROW)
# per-(ffn co) rows: col = 128 + co*2 + row
CFK, CFRN = range(2)
ONES_COL = 192           # 1.0 f32 (bitcast f32r for ones-matmul lhsT)
# carries tile [128, CB, 4] bf16 rows
CAR_U, CAR_U2, CAR_A, CAR_B = range(4)


def _bcast_free(ap, n):
    """[128,1] AP -> [128,n] stride-0 broadcast along free dim."""
    import concourse.bass as bass
    return bass.AP(tensor=ap.tensor, offset=ap.offset, ap=[ap.ap[0], [0, n]])


def _bcast_mid(ap, nmid):
    """[128,N] AP -> [128,nmid,N] stride-0 broadcast of a middle dim."""
    import concourse.bass as bass
    return bass.AP(tensor=ap.tensor, offset=ap.offset,
                   ap=[ap.ap[0], [0, nmid], ap.ap[1]])


def _pair_shift(t, a, n):
    """u-tile [128, CB, n+1] -> [128, 2, n] AP at block a: [p, i, t] =
    u[p, a, i + t]  (i=0: shifted/prev token, i=1: current token)."""
    import concourse.bass as bass
    ap = t[:, a, :]
    return bass.AP(tensor=ap.tensor, offset=ap.offset,
                   ap=[ap.ap[0], [1, 2], [1, n]])


def _build():
    import concourse.bass as bass
    import concourse.bacc as bacc
    import concourse.tile as tile
    import contextlib
    from concourse import mybir

    f32 = mybir.dt.float32
    f32r = mybir.dt.float32r
    bf16 = mybir.dt.bfloat16
    fp8 = mybir.dt.float8e4
    AF = mybir.ActivationFunctionType
    OP = mybir.AluOpType
    DR = mybir.MatmulPerfMode.DoubleRow

    nc = bacc.Bacc(None, target_bir_lowering=False, debug=False)

    xT = nc.dram_tensor("xT", [C, T], f32r, kind="ExternalInput")
    cvd = nc.dram_tensor("cvall", [128, CVW], f32r, kind="ExternalInput")
    ones16_in = nc.dram_tensor("ones128b", [128], bf16, kind="ExternalInput")
    ones_bin = nc.dram_tensor("onesb", [128], f32r, kind="ExternalInput")
    Wk2 = nc.dram_tensor("Wk2", [C, 2, C], fp8, kind="ExternalInput")
    Wv2 = nc.dram_tensor("Wv2", [C, 2, C], fp8, kind="ExternalInput")
    Wr2 = nc.dram_tensor("Wr2", [C, 2, C], fp8, kind="ExternalInput")
    Wo8 = nc.dram_tensor("Wo8", [C, C], fp8, kind="ExternalInput")
    fWk16 = nc.dram_tensor("fWk16", [C, 4 * C], bf16, kind="ExternalInput")
    fWr8 = nc.dram_tensor("fWr8", [C, C], fp8, kind="ExternalInput")
    fWv16 = nc.dram_tensor("fWv16", [4 * C, C], bf16, kind="ExternalInput")
    outT = nc.dram_tensor("outT", [C, T], f32, kind="ExternalOutput")

    xTr = xT.rearrange("(cb p) t -> p cb t", p=128)
    outTr = outT.rearrange("(cb p) t -> p cb t", p=128)

    with tile.TileContext(nc) as tc:
      with contextlib.ExitStack() as ctx:
        consts = ctx.enter_context(tc.tile_pool(name="consts", bufs=1))
        dramp = ctx.enter_context(tc.tile_pool(name="dram", bufs=1, space="DRAM"))

        cvt = consts.tile([128, CVW], f32r)
        nc.sync.dma_start(out=cvt, in_=cvd[:, :])
        cvtf = cvt.bitcast(f32)
        ones_b = consts.tile([1, 128], f32r)
        nc.sync.dma_start(out=ones_b, in_=ones_bin.rearrange("(o p) -> o p", o=1))
        ones_k16 = consts.tile([128, 1], bf16)
        nc.sync.dma_start(out=ones_k16, in_=ones16_in.rearrange("(p o) -> p o", o=1))
        ones_k = cvt[:, ONES_COL:ONES_COL + 1]

        def cva(cb, row):
            i = cb * NROW + row
            return cvtf[:, i:i + 1]

        def cvf(co, row):
            i = 128 + co * 2 + row
            return cvtf[:, i:i + 1]

        car = consts.tile([128, CB, 4], bf16)
        nc.vector.tensor_copy(out=car[:, :, CAR_U:CAR_U + 1],
                              in_=cvtf[:, 0:128].rearrange(
                                  "p (cb r) -> p cb r", r=NROW)[:, :, U1INIT:U1INIT + 1])
        nc.vector.tensor_copy(out=car[:, :, CAR_U2:CAR_U2 + 1],
                              in_=cvtf[:, 0:128].rearrange(
                                  "p (cb r) -> p cb r", r=NROW)[:, :, U2INIT:U2INIT + 1])
        nc.vector.memset(car[:, :, CAR_A:CAR_B + 1], 0.0)

        x2d = dramp.tile([NCH, 128, CB, TC], f32)
        kkd = dramp.tile([NCH, 128, FB, TC], bf16, tag="kkd")
        rrd = dramp.tile([NCH, 128, CB, TC], bf16, tag="rrd")

        def layernorm_stats(pools, x_t, sq16, rows, tmp):
            """Per-token mean + 16*rstd rows from x_t [128,CB,TC] f32.

            rows[:,0,:]=m  rows[:,1,:]=16*rstd (f32r-typed for the broadcast
            matmuls); rstd = exp(-0.5*ln(var+eps)+ln(16)) stays on the exp/ln
            activation table.
            """
            ps_stx, ps_stq = pools
            nc.scalar.activation(out=sq16, in_=x_t.bitcast(f32), func=AF.Square)
            for cb in range(CB):
                nc.tensor.matmul(ps_stx, ones_k, x_t[:, cb, :],
                                 start=(cb == 0), stop=(cb == CB - 1))
            for cb in range(CB):
                nc.tensor.matmul(ps_stq, ones_k16, sq16[:, cb, :],
                                 start=(cb == 0), stop=(cb == CB - 1))
            rowf = rows.bitcast(f32)
            nc.vector.tensor_scalar_mul(rows[:, 0, :], ps_stx, 1.0 / C)
            nc.vector.tensor_mul(tmp[:, 0, :], rowf[:, 0, :], rowf[:, 0, :])
            nc.vector.scalar_tensor_tensor(
                out=tmp[:, 1, :], in0=ps_stq, scalar=1.0 / C,
                in1=tmp[:, 0, :], op0=OP.mult, op1=OP.subtract)
            nc.scalar.activation(out=tmp[:, 0, :], in_=tmp[:, 1, :],
                                 func=AF.Ln, bias=cvtf[0:1, EPSR + 0:EPSR + 1])
            nc.scalar.activation(out=tmp[:, 1, :], in_=tmp[:, 0, :],
                                 func=AF.Exp, scale=-0.5,
                                 bias=cvtf[0:1, LNSCR:LNSCR + 1])
            nc.vector.tensor_copy(out=rows[:, 1, :], in_=tmp[:, 1, :])

        # ================= Phase 1: fused time-mix =================
        with contextlib.ExitStack() as p1:
            wpool = p1.enter_context(tc.tile_pool(name="w1", bufs=1))
            dbl = p1.enter_context(tc.tile_pool(name="dbl1", bufs=2))
            sgl = p1.enter_context(tc.tile_pool(name="sgl1", bufs=1))
            rowp = p1.enter_context(tc.tile_pool(name="rows1", bufs=1))
            ps_mm = p1.enter_context(tc.tile_pool(name="ps_mm", bufs=3, space="PSUM"))
            ps_st = p1.enter_context(tc.tile_pool(name="ps_st", bufs=1, space="PSUM"))
            ps_bc = p1.enter_context(tc.tile_pool(name="ps_bc", bufs=1, space="PSUM"))

            def prep1(ic):
                """DMA + LN + u8p input prep for chunk ic (pipelined ahead)."""
                t0 = ic * TC
                x_t = dbl.tile([128, CB, TC], f32r, tag="x")
                nc.sync.dma_start(out=x_t, in_=xTr[:, :, t0:t0 + TC])
                x_f = x_t.bitcast(f32)
                sq16 = sgl.tile([128, CB, TC], bf16, tag="sq")
                rows = rowp.tile([1, 2, TC], f32r, tag="rows")
                tmp = rowp.tile([1, 2, TC], f32, tag="rtmp")
                ps_stx = ps_st.tile([1, TC], f32, tag="stx")
                ps_stq = ps_st.tile([1, TC], f32, tag="stq")
                layernorm_stats((ps_stx, ps_stq), x_t, sq16, rows, tmp)
                bc = ps_bc.tile([128, 2, TC], f32, tag="bc")
                nc.tensor.matmul(bc[:, 0, :], ones_b, rows[:, 0, :])
                nc.tensor.matmul(bc[:, 1, :], ones_b, rows[:, 1, :])
                s1 = sgl.tile([128, CB, TC], f32, tag="s1")
                nc.vector.tensor_sub(s1, x_f, _bcast_mid(bc[:, 0, :], CB))
                # u8p[:, cb, 1, :] = current token (16*u); [:, cb, 0, :] = prev
                u8p = dbl.tile([128, CB, 2, TC], fp8, tag="u8")
                nc.vector.tensor_mul(u8p[:, :, 1, :], s1,
                                     _bcast_mid(bc[:, 1, :], CB))
                nc.vector.tensor_copy(out=u8p[:, :, 0, 0:1],
                                      in_=car[:, :, CAR_U:CAR_U + 1])
                nc.vector.tensor_copy(out=u8p[:, :, 0, 1:TC],
                                      in_=u8p[:, :, 1, 0:TC - 1])
                nc.vector.tensor_copy(out=car[:, :, CAR_U:CAR_U + 1],
                                      in_=u8p[:, :, 1, TC - 1:TC])
                return x_f, u8p

            state = prep1(0)

            wk_t = wpool.tile([128, CB, 2, C], fp8, tag="wk")
            wv_t = wpool.tile([128, CB, 2, C], fp8, tag="wv")
            wr_t = wpool.tile([128, CB, 2, C], fp8, tag="wr")
            wo_t = wpool.tile([128, CB, C], fp8, tag="wo")
            for wt, wd in ((wk_t, Wk2), (wv_t, Wv2), (wr_t, Wr2)):
                wre = wd.rearrange("(a p) i m -> p a i m", p=128)
                for h in range(2):
                    nc.sync.dma_start(out=wt[:, 4 * h:4 * h + 4], in_=wre[:, 4 * h:4 * h + 4])
            nc.sync.dma_start(out=wo_t, in_=Wo8.rearrange("(a p) m -> p a m", p=128))

            for ic in range(NCH):
                x_f, u8p = state
                ek = sgl.tile([128, CB, TC], bf16, tag="ek")
                ekv = sgl.tile([128, CB, TC], bf16, tag="ekv")
                e_r = sgl.tile([128, CB, TC], bf16, tag="er")
                for wt, post in (
                    (wk_t, lambda co, ps: nc.scalar.activation(
                        out=ek[:, co, :], in_=ps, func=AF.Exp,
                        scale=cva(0, SCK), bias=cva(co, CK))),
                    (wv_t, lambda co, ps: nc.scalar.activation(
                        out=ekv[:, co, :], in_=ps, func=AF.Identity,
                        scale=cva(0, SCV), bias=cva(co, CV32))),
                    (wr_t, lambda co, ps: nc.scalar.activation(
                        out=e_r[:, co, :], in_=ps, func=AF.Exp,
                        scale=cva(0, SCRN), bias=cva(co, CRN))),
                ):
                    for co in range(CB):
                        ps = ps_mm.tile([128, TC], f32, tag="mm")
                        csl = slice(co * 128, (co + 1) * 128)
                        for a in range(CB):
                            nc.tensor.matmul(ps, wt[:, a, :, csl],
                                             u8p[:, a, :, :],
                                             start=(a == 0), stop=(a == CB - 1),
                                             perf_mode=DR)
                        post(co, ps)

                if ic + 1 < NCH:
                    state = prep1(ic + 1)

                # ekv = (32*v) * exp(k)   (in place)
                nc.vector.tensor_mul(ekv, ekv, ek)

                AB = sgl.tile([128, 2, CB, TC + 1], bf16, tag="AB")
                nc.vector.tensor_copy(out=AB[:, 0, :, 0:1], in_=car[:, :, CAR_A:CAR_A + 1])
                nc.vector.tensor_copy(out=AB[:, 1, :, 0:1], in_=car[:, :, CAR_B:CAR_B + 1])
                for cb in range(CB):
                    ew_b = _bcast_free(cva(cb, EW), TC)
                    nc.vector.tensor_tensor_scan(
                        out=AB[:, 0, cb, 1:TC + 1], data0=ew_b, data1=ekv[:, cb, :],
                        initial=AB[:, 0, cb, 0:1], op0=OP.mult, op1=OP.add)
                    nc.vector.tensor_tensor_scan(
                        out=AB[:, 1, cb, 1:TC + 1], data0=ew_b, data1=ek[:, cb, :],
                        initial=AB[:, 1, cb, 0:1], op0=OP.mult, op1=OP.add)
                nc.vector.tensor_copy(out=car[:, :, CAR_A:CAR_A + 1],
                                      in_=AB[:, 0, :, TC:TC + 1])
                nc.vector.tensor_copy(out=car[:, :, CAR_B:CAR_B + 1],
                                      in_=AB[:, 1, :, TC:TC + 1])

                # num -> ekv, den -> ek (in place)
                for cb in range(CB):
                    eu_s = cva(cb, EU)
                    nc.vector.scalar_tensor_tensor(
                        out=ekv[:, cb, :], in0=ekv[:, cb, :], scalar=eu_s,
                        in1=AB[:, 0, cb, 0:TC], op0=OP.mult, op1=OP.add)
                    nc.vector.scalar_tensor_tensor(
                        out=ek[:, cb, :], in0=ek[:, cb, :], scalar=eu_s,
                        in1=AB[:, 1, cb, 0:TC], op0=OP.mult, op1=OP.add)
                # den2 = den * (1 + e_r): folds the r-sigmoid into the division
                den2 = sgl.tile([128, CB, TC], f32, tag="s1")
                nc.vector.scalar_tensor_tensor(
                    out=den2, in0=e_r, scalar=1.0, in1=ek,
                    op0=OP.add, op1=OP.mult)
                nc.vector.reciprocal_approx_fast(out=den2, in_=den2)
                y8 = sgl.tile([128, CB, TC], fp8, tag="er")
                nc.vector.tensor_mul(y8, ekv, den2)

                x2 = sgl.tile([128, CB, TC], f32, tag="x2")
                for co in range(CB):
                    ps = ps_mm.tile([128, TC], f32, tag="mm")
                    csl = slice(co * 128, (co + 1) * 128)
                    for j in range(CB // 2):
                        nc.tensor.matmul(ps, wo_t[:, 2 * j:2 * j + 2, csl],
                                         y8[:, 2 * j:2 * j + 2, :],
                                         start=(j == 0), stop=(j == CB // 2 - 1),
                                         perf_mode=DR)
                    nc.vector.scalar_tensor_tensor(
                        out=x2[:, co, :], in0=ps, scalar=cva(0, SCO),
                        in1=x_f[:, co, :], op0=OP.mult, op1=OP.add)
                nc.sync.dma_start(out=x2d[ic], in_=x2)

        # ================= Phase 2a: FFN up (kk + rr) =================
        with contextlib.ExitStack() as p2:
            wpool = p2.enter_context(tc.tile_pool(name="w2", bufs=1))
            dbl = p2.enter_context(tc.tile_pool(name="dbl2", bufs=2))
            sgl = p2.enter_context(tc.tile_pool(name="sgl2", bufs=1))
            rowp = p2.enter_context(tc.tile_pool(name="rows2", bufs=1))
            ps_mm = p2.enter_context(tc.tile_pool(name="ps_mm2", bufs=3, space="PSUM"))
            ps_st = p2.enter_context(tc.tile_pool(name="ps_st2", bufs=1, space="PSUM"))
            ps_bc = p2.enter_context(tc.tile_pool(name="ps_bc2", bufs=1, space="PSUM"))

            def prep2(ic):
                x2_t = dbl.tile([128, CB, TC], f32r, tag="x2i")
                nc.sync.dma_start(out=x2_t, in_=x2d[ic].bitcast(f32r))
                x2_f = x2_t.bitcast(f32)
                sq16 = sgl.tile([128, CB, TC], bf16, tag="sq2")
                rows = rowp.tile([1, 2, TC], f32r, tag="rows")
                tmp = rowp.tile([1, 2, TC], f32, tag="rtmp")
                ps_stx = ps_st.tile([1, TC], f32, tag="stx")
                ps_stq = ps_st.tile([1, TC], f32, tag="stq")
                layernorm_stats((ps_stx, ps_stq), x2_t, sq16, rows, tmp)
                bc = ps_bc.tile([128, 2, TC], f32, tag="bc")
                nc.tensor.matmul(bc[:, 0, :], ones_b, rows[:, 0, :])
                nc.tensor.matmul(bc[:, 1, :], ones_b, rows[:, 1, :])
                s1 = sgl.tile([128, CB, TC], f32, tag="s12")
                nc.vector.tensor_sub(s1, x2_f, _bcast_mid(bc[:, 0, :], CB))
                u2 = sgl.tile([128, CB, TC + 1], bf16, tag="u2")
                nc.vector.tensor_copy(out=u2[:, :, 0:1], in_=car[:, :, CAR_U2:CAR_U2 + 1])
                nc.vector.tensor_mul(u2[:, :, 1:TC + 1], s1,
                                     _bcast_mid(bc[:, 1, :], CB))
                nc.vector.tensor_copy(out=car[:, :, CAR_U2:CAR_U2 + 1],
                                      in_=u2[:, :, TC:TC + 1])
                dif = sgl.tile([128, CB, TC], bf16, tag="sq2")
                nc.vector.tensor_sub(dif, u2[:, :, 1:TC + 1], u2[:, :, 0:TC])
                w16 = sgl.tile([128, CB, TC], bf16, tag="w16")
                fin_r8 = sgl.tile([128, CB, TC], fp8, tag="finr")
                for cb in range(CB):
                    nc.vector.scalar_tensor_tensor(
                        out=w16[:, cb, :], in0=dif[:, cb, :],
                        scalar=cva(cb, FTMK),
                        in1=u2[:, cb, 0:TC], op0=OP.mult, op1=OP.add)
                    nc.vector.scalar_tensor_tensor(
                        out=fin_r8[:, cb, :], in0=dif[:, cb, :],
                        scalar=cva(cb, FTMR),
                        in1=u2[:, cb, 0:TC], op0=OP.mult, op1=OP.add)
                return w16, fin_r8

            state = prep2(0)

            fwk_t = wpool.tile([128, CB, 4 * C], bf16, tag="fwk")
            fwkr = fWk16.rearrange("(a p) m -> p a m", p=128)
            for q in range(4):
                nc.sync.dma_start(out=fwk_t[:, :, q * C:(q + 1) * C],
                                  in_=fwkr[:, :, q * C:(q + 1) * C])
            fwr_t = wpool.tile([128, CB, C], fp8, tag="fwr")
            nc.sync.dma_start(out=fwr_t, in_=fWr8.rearrange("(a p) m -> p a m", p=128))

            for ic in range(NCH):
                w16, fin_r8 = state
                for q in range(4):
                    rt = sgl.tile([128, CB, TC], bf16, tag="rt")
                    for j in range(CB):
                        co = q * CB + j
                        ps = ps_mm.tile([128, TC], f32, tag="mm")
                        csl = slice(co * 128, (co + 1) * 128)
                        for a in range(CB):
                            nc.tensor.matmul(ps, fwk_t[:, a, csl], w16[:, a, :],
                                             start=(a == 0), stop=(a == CB - 1))
                        nc.scalar.activation(out=rt[:, j, :], in_=ps, func=AF.Relu,
                                             bias=cvf(co, CFK))
                    kkq = dbl.tile([128, CB, TC], bf16, tag="kkq")
                    nc.scalar.activation(out=kkq, in_=rt, func=AF.Square)
                    nc.sync.dma_start(out=kkd[ic][:, q * CB:(q + 1) * CB, :], in_=kkq)

                e_rr = sgl.tile([128, CB, TC], bf16, tag="rt")
                for co in range(CB):
                    ps = ps_mm.tile([128, TC], f32, tag="mm")
                    csl = slice(co * 128, (co + 1) * 128)
                    for j in range(CB // 2):
                        nc.tensor.matmul(ps, fwr_t[:, 2 * j:2 * j + 2, csl],
                                         fin_r8[:, 2 * j:2 * j + 2, :],
                                         start=(j == 0), stop=(j == CB // 2 - 1),
                                         perf_mode=DR)
                    nc.scalar.activation(out=e_rr[:, co, :], in_=ps, func=AF.Exp,
                                         scale=cva(0, SCFR),
                                         bias=cvf(co, CFRN))

                if ic + 1 < NCH:
                    state = prep2(ic + 1)

                den = sgl.tile([128, CB, TC], f32, tag="s12")
                nc.vector.tensor_scalar_add(out=den, in0=e_rr, scalar1=1.0)
                nc.vector.reciprocal_approx_fast(out=den, in_=den)
                rr16 = sgl.tile([128, CB, TC], bf16, tag="rr")
                nc.vector.tensor_copy(out=rr16, in_=den)
                nc.sync.dma_start(out=rrd[ic], in_=rr16)

        # ================= Phase 2b: FFN down + residual =================
        with contextlib.ExitStack() as p3:
            wpool = p3.enter_context(tc.tile_pool(name="w3", bufs=1))
            dbl = p3.enter_context(tc.tile_pool(name="dbl3", bufs=2))
            halfp = p3.enter_context(tc.tile_pool(name="half3", bufs=3))
            ps_mm = p3.enter_context(tc.tile_pool(name="ps_mm3", bufs=4, space="PSUM"))

            def prep3(ic):
                kk0 = halfp.tile([128, FB // 2, TC], bf16, tag="kkh")
                nc.sync.dma_start(out=kk0, in_=kkd[ic][:, 0:FB // 2, :])
                kk1 = halfp.tile([128, FB // 2, TC], bf16, tag="kkh")
                nc.sync.dma_start(out=kk1, in_=kkd[ic][:, FB // 2:FB, :])
                rr16 = dbl.tile([128, CB, TC], bf16, tag="rri")
                nc.sync.dma_start(out=rr16, in_=rrd[ic])
                x2_t = dbl.tile([128, CB, TC], f32, tag="x2b")
                nc.sync.dma_start(out=x2_t, in_=x2d[ic])
                return kk0, kk1, rr16, x2_t

            state = prep3(0)

            fwv_t = wpool.tile([128, FB, C], bf16, tag="fwv")
            fwvr = fWv16.rearrange("(a p) m -> p a m", p=128)
            for s in range(4):
                nc.sync.dma_start(out=fwv_t[:, s * 8:(s + 1) * 8, :],
                                  in_=fwvr[:, s * 8:(s + 1) * 8, :])

            for ic in range(NCH):
                t0 = ic * TC
                kk0, kk1, rr16, x2_t = state
                if ic + 1 < NCH:
                    state = prep3(ic + 1)

                out_t = dbl.tile([128, CB, TC], f32, tag="out")
                for co in range(CB):
                    ps = ps_mm.tile([128, TC], f32, tag="mm")
                    csl = slice(co * 128, (co + 1) * 128)
                    for a in range(FB):
                        kkh = kk0 if a < FB // 2 else kk1
                        nc.tensor.matmul(ps, fwv_t[:, a, csl],
                                         kkh[:, a % (FB // 2), :],
                                         start=(a == 0), stop=(a == FB - 1))
                    nc.vector.tensor_mul(ps, rr16[:, co, :], ps)
                    nc.vector.tensor_add(out_t[:, co, :], x2_t[:, co, :], ps)
                nc.sync.dma_start(out=outTr[:, :, t0:t0 + TC], in_=out_t)

    nc.finalize()
    return nc


def _prep_maps(inputs):
    E4 = ml_dtypes.float8_e4m3
    BF = ml_dtypes.bfloat16
    f32 = np.float32

    x = np.asarray(inputs["x"], f32)
    ln1_g = np.asarray(inputs["ln1_g"], f32)
    ln1_b = np.asarray(inputs["ln1_b"], f32)
    ln2_g = np.asarray(inputs["ln2_g"], f32)
    ln2_b = np.asarray(inputs["ln2_b"], f32)
    tmk = np.asarray(inputs["tmk"], f32)
    tmv = np.asarray(inputs["tmv"], f32)
    tmr = np.asarray(inputs["tmr"], f32)
    ftmk = np.asarray(inputs["ftmk"], f32)
    ftmr = np.asarray(inputs["ftmr"], f32)
    ew = np.exp(-np.exp(np.asarray(inputs["time_decay"], f32))).astype(f32)
    eu = np.exp(np.asarray(inputs["time_first"], f32)).astype(f32)

    def pow2s(m, target=224.0):
        m = float(m)
        if m <= 0:
            return 1.0
        return 2.0 ** math.floor(math.log2(target / m))

    def fold2(W, tm, g, b):
        """Pack [(1-tm)*g*W | tm*g*W] * S as [C, 2, Co] e4m3; const = b@W."""
        W = np.asarray(W, f32)
        Wa = ((1.0 - tm) * g)[:, None] * W
        Wb = (tm * g)[:, None] * W
        S = pow2s(max(np.abs(Wa).max(), np.abs(Wb).max()))
        P = np.stack([Wa, Wb], axis=1) * S
        return np.ascontiguousarray(P).astype(E4), S, (b @ W).astype(f32)

    Wk2, Sk, ck = fold2(inputs["Wk"], tmk, ln1_g, ln1_b)
    Wv2, Sv, cv_ = fold2(inputs["Wv"], tmv, ln1_g, ln1_b)
    Wr2, Sr, cr_ = fold2(inputs["Wr"], tmr, ln1_g, ln1_b)

    Wo = np.asarray(inputs["Wo"], f32)
    So = pow2s(np.abs(Wo).max())
    Wo8 = (Wo * So).astype(E4)

    fWk = np.asarray(inputs["fWk"], f32)
    fWk16 = (ln2_g[:, None] * fWk / SU).astype(BF)
    cfk = (ln2_b @ fWk).astype(f32)

    fWr = np.asarray(inputs["fWr"], f32)
    fWrg = ln2_g[:, None] * fWr
    Sfr = pow2s(np.abs(fWrg).max())
    fWr8 = (fWrg * Sfr).astype(E4)
    cfr = (ln2_b @ fWr).astype(f32)

    fWv16 = np.asarray(inputs["fWv"], f32).astype(BF)

    def plane(vec):
        # [C] indexed by channel -> [128, CB] (p, cb)
        return np.ascontiguousarray(np.asarray(vec, f32).reshape(CB, 128).T)

    with np.errstate(divide="ignore", invalid="ignore"):
        u1i = np.where(ln1_g != 0, -SU * ln1_b / ln1_g, 0.0)
        u2i = np.where(ln2_g != 0, -SU * ln2_b / ln2_g, 0.0)
    u1i = np.clip(np.nan_to_num(u1i), -400, 400)
    u2i = np.clip(np.nan_to_num(u2i), -400, 400)

    rows = np.zeros((128, CB, NROW), f32)
    rows[:, :, EW] = plane(ew)
    rows[:, :, EU] = plane(eu)
    rows[:, :, CK] = plane(ck)
    rows[:, :, CV32] = plane(SV * cv_)
    rows[:, :, CRN] = plane(-cr_)
    rows[:, :, U1INIT] = plane(u1i)
    rows[:, :, U2INIT] = plane(u2i)
    rows[:, :, FTMK] = plane(ftmk)
    rows[:, :, FTMR] = plane(ftmr)
    rows[:, :, SCK] = 1.0 / (SU * Sk)
    rows[:, :, SCV] = SV / (SU * Sv)
    rows[:, :, SCRN] = -1.0 / (SU * Sr)
    rows[:, :, SCO] = 1.0 / (SV * So)
    rows[:, :, SCFR] = -1.0 / (SU * Sfr)
    rows[:, :, EPSR] = EPS
    rows[:, :, LNSCR] = math.log(SU)

    cvall = np.zeros((128, CVW), f32)
    cvall[:, 0:128] = rows.reshape(128, 128)
    ffn = np.zeros((128, FB, 2), f32)
    ffn[:, :, CFK] = np.ascontiguousarray(cfk.reshape(FB, 128).T)
    ffn[:, 0:CB, CFRN] = np.ascontiguousarray((-cfr).reshape(CB, 128).T)
    cvall[:, 128:192] = ffn.reshape(128, 64)
    cvall[:, ONES_COL] = 1.0

    common = {
        "cvall": cvall,
        "ones128b": np.ones(128, BF), "onesb": np.ones(128, f32),
        "Wk2": Wk2, "Wv2": Wv2, "Wr2": Wr2, "Wo8": Wo8,
        "fWk16": fWk16, "fWr8": fWr8, "fWv16": fWv16,
    }
    return [{**common, "xT": np.ascontiguousarray(x[b].T)} for b in range(B)]


def get_nc():
    if "nc" not in _CACHE:
        _CACHE["nc"] = _build()
    return _CACHE["nc"]


def kernel(**inputs):
    from concourse.bass_utils import run_bass_kernel_spmd
    nc = get_nc()
    in_maps = _prep_maps(inputs)
    res = run_bass_kernel_spmd(nc, in_maps, core_ids=list(range(B)))
    return np.stack([np.ascontiguousarray(r["outT"].T) for r in res.results])
